# revision 46
# baseline (speedup 1.0000x reference)
"""Trainium2 Bass kernel for CustomSelfAttention (B=8,S=1024,D=1024,H=16,K=64).

Strategy: data-parallel over batch across 8 NeuronCores (1 batch item/core).
All matmuls in bf16. Host prep does everything the device is bad at:
transposes x to xT [d, s] (the on-chip XBAR transpose is a ~110GB/s global
bottleneck), folds 1/sqrt(K) into Wq, computes the uniform-attention row
u = (mean_s(x) Wv + bv) Wo + bo exactly, and upcasts the bf16 output.

Per-core pipeline (HW exec ~250us, ~90% of the practical floor):
  0. xT + wv land as 256KB chunks interleaved across both DGE queues
     (~170GB/s shared); small consts as linear row DMAs + Pool broadcasts
     (broadcast DMAs are descriptor storms that starve the trickle).
  1. v-projection: the first 6 PSUM accumulator groups are spread over
     pmm+pscore+pctx and emitted dc-major so each arriving chunk releases
     6 matmuls — the PE rides the DMA trickle ~70% busy and the HAM clock
     (warmed by a few scratch dummy matmuls) never re-throttles to 1.2GHz.
     Then qT = (Wq/8)^T xT, kT = Wk^T xT [hk, s]; v stored with a ones
     column per head: vext [s, h, 65].
  2. attention per head PAIR (even head on PE rows 0-63, odd on 64-127 via
     row tiling -> the two K=64 scores matmuls run concurrently, +4ns);
     one Exp over a 2-bank PSUM tile [128,1024] with the per-partition
     key-mask bias fused; ctx matmuls with lhsT=[v_h | 1] give ctx^T[k,q]
     plus softmax row sums in one shot; normalize with
     reciprocal_approx_fast + partition_broadcast. QKV projection matmuls
     for chunk c+1 interleave into attention chunk c's PE queue to cover
     the ACT-bound exp latency (ACT exp floor: 147us).
  3. out = ctxT^T Wo blended with u for masked queries:
     out = po*mq + u*(1-mq). The final 8 output tiles pre-accumulate
     their c=0..6 matmuls across the last normalization chain, rotate po
     tiles over pmm+pscore, compute the blend products on the (then idle)
     ACT engine, and DMA out in bf16.
"""

import contextlib
import sys
import types

sys.path.insert(0, "/opt/trn_rl_repo")

# The image's antenv package may lack axon_hooks (NTFF profile hook
# registry); bass_utils imports it unconditionally when trace=True.
# Install a functional shim + register the ctypes hook like
# trn_agent_boot.trn_boot does.
try:
    import antenv.axon_hooks  # noqa: F401
except ImportError:
    try:
        import antenv

        _hooks_mod = types.ModuleType("antenv.axon_hooks")
        _hook_box = [None]
        _hooks_mod.get_axon_ntff_profile_hook = lambda: _hook_box[0]
        _hooks_mod.set_axon_ntff_profile_hook = (
            lambda h: _hook_box.__setitem__(0, h)
        )
        sys.modules["antenv.axon_hooks"] = _hooks_mod
        antenv.axon_hooks = _hooks_mod
        from trn_agent_boot.trn_boot import _ntff_profile_via_ctypes

        _hooks_mod.set_axon_ntff_profile_hook(
            _ntff_profile_via_ctypes("/opt/axon/libaxon_pjrt.so")
        )
    except Exception:
        pass

import ml_dtypes  # noqa: E402
import numpy as np  # noqa: E402

import concourse.bass as bass  # noqa: E402
import concourse.bass_utils as _bass_utils  # noqa: E402
import concourse.mybir as mybir  # noqa: E402
import concourse.tile as tile  # noqa: E402
from concourse import bacc  # noqa: E402
from concourse.bass_utils import run_bass_kernel_spmd  # noqa: E402
from concourse.masks import make_identity  # noqa: E402

# Enable the walrus LDWEIGHTS background-buffer optimization for this
# kernel's compile: without it every MATMUL serializes behind its
# foreground weight load (~+170ns per matmul on this kernel). Walrus
# rejects ldw-opt when LDWEIGHTS carry semaphore waits, so the bass pass
# that moves matmul waits onto LDWEIGHTS must be skipped too (see
# _build_nc).
LDW_OPT = False

if not getattr(_bass_utils, "_ldwopt_patched", False):
    _orig_run_command = _bass_utils.run_command

    def _run_command_ldwopt(argv, **kwargs):
        if LDW_OPT and isinstance(argv, list):
            argv = [
                "--enable-ldw-opt=true" if a == "--enable-ldw-opt=false" else a
                for a in argv
            ]
        return _orig_run_command(argv, **kwargs)

    _bass_utils.run_command = _run_command_ldwopt
    _bass_utils._ldwopt_patched = True

F32 = mybir.dt.float32
BF16 = mybir.dt.bfloat16
AF = mybir.ActivationFunctionType
OP = mybir.AluOpType

B, S, D, H, K = 8, 1024, 1024, 16, 64
HK = H * K
P = 128
SC = S // P      # 8 s-chunks
DC = D // P      # 8 d-chunks
HKC = HK // P    # 8 hk-chunks (head pairs)
NQW = S // 512   # 2 q-windows of 512
NEG = -1e9

TRACE = False  # set by test.py for profiling runs

_nc_cache = None


def _build_nc(repeat=1):
    nc = bacc.Bacc(None, target_bir_lowering=False)
    if LDW_OPT:
        # leave waits on the matmuls; walrus ldw-opt refuses LDWEIGHTS
        # that carry semaphore waits
        nc.move_matmul_waits_to_ldweights = lambda: None

    # x arrives HOST-TRANSPOSED as xT [D, S] (numpy .T in _prep_in_maps):
    # the on-chip XBAR DMA-transpose is a ~110 GB/s GLOBAL bottleneck
    # (measured across both queues), so 2MB of x gated the whole start by
    # ~18us; linear chunked loads of a pre-transposed x remove it.
    x_d = nc.dram_tensor("x", [D, S], BF16, kind="ExternalInput")
    wq_d = nc.dram_tensor("wq", [D, HK], BF16, kind="ExternalInput")
    wk_d = nc.dram_tensor("wk", [D, HK], BF16, kind="ExternalInput")
    wv_d = nc.dram_tensor("wv", [D, HK], BF16, kind="ExternalInput")
    wo_d = nc.dram_tensor("wo", [HK, D], BF16, kind="ExternalInput")
    bq_d = nc.dram_tensor("bq", [HK], F32, kind="ExternalInput")
    bk_d = nc.dram_tensor("bk", [HK], F32, kind="ExternalInput")
    bv_d = nc.dram_tensor("bv", [HK], F32, kind="ExternalInput")
    # u = (mean_s(x) @ Wv + bv) @ Wo + bo, computed EXACTLY on the host
    # (all inputs known): kills the on-device u-path (~42 PE steps)
    u_d = nc.dram_tensor("u", [D], F32, kind="ExternalInput")
    ka_d = nc.dram_tensor("ka", [S], F32, kind="ExternalInput")   # (m-1)*1e9
    mq_d = nc.dram_tensor("mq", [S], F32, kind="ExternalInput")   # mask 0/1
    om_d = nc.dram_tensor("om", [S], F32, kind="ExternalInput")   # 1-mask
    # output in bf16 (host upcasts to f32): halves the 4MB output DMA
    out_d = nc.dram_tensor("out", [S, D], BF16, kind="ExternalOutput")

    def bcast_ap(t, counts, step_last=None):
        # DRAM AP broadcasting a small tensor across leading 0-stride dims.
        ap = [[0, c] for c in counts]
        ap.append(step_last if step_last is not None else [1, 1])
        return bass.AP(tensor=t, offset=0, ap=ap)

    with tile.TileContext(nc) as tc:
        with (
            tc.tile_pool(name="consts", bufs=1) as consts,
            tc.tile_pool(name="big", bufs=1) as big,
            tc.tile_pool(name="wpool", bufs=1) as wpool,
            tc.tile_pool(name="epool", bufs=6) as epool,
            tc.tile_pool(name="rb", bufs=2) as rbpool,
            tc.tile_pool(name="rp", bufs=4) as rpool,
            tc.tile_pool(name="cn", bufs=2) as cnpool,
            tc.tile_pool(name="op", bufs=4) as opool,
            tc.tile_pool(name="dram", bufs=1, space="DRAM") as drampool,
            tc.tile_pool(name="pmm", bufs=2, space="PSUM") as pmm,
            tc.tile_pool(name="pscore", bufs=2, space="PSUM") as pscore,
            tc.tile_pool(name="pctx", bufs=2, space="PSUM") as pctx,
        ):
            # ---- constant tiles (DMAs emitted in _emit_body AFTER the
            # x transposes so they don't block the SP queue at t=0) ----
            ka_sb = consts.tile([P, SC], F32)
            mq_sb = consts.tile([P, SC], F32)
            om_sb = consts.tile([P, SC], F32)
            bq_sb = consts.tile([P, HKC], F32)
            bk_sb = consts.tile([P, HKC], F32)
            bv_row = consts.tile([1, HK], F32)
            bv_bc = consts.tile([P, HK], F32)
            u_row = consts.tile([1, D], F32)
            u_bc = consts.tile([P, D], F32)
            scratch = consts.tile([P, 512], BF16)

            loop_cm = (
                tc.For_i(
                    0,
                    repeat,
                    1,
                    hint_engines=(
                        mybir.EngineType.PE,
                        mybir.EngineType.Activation,
                        mybir.EngineType.DVE,
                        mybir.EngineType.SP,
                        mybir.EngineType.Pool,
                    ),
                )
                if repeat > 1
                else contextlib.nullcontext()
            )
            with loop_cm:
                _emit_body(
                    nc, tc, x_d, wq_d, wk_d, wv_d, wo_d, out_d, bcast_ap,
                    ka_sb, mq_sb, om_sb, bq_sb, bk_sb, bv_row, bv_bc,
                    u_row, u_bc, scratch, consts, big, wpool, epool, rbpool,
                    rpool, cnpool, opool, drampool, pmm, pscore, pctx,
                    ka_d, mq_d, om_d, bq_d, bk_d, bv_d, u_d,
                )

    nc.compile()
    return nc


def _emit_body(
    nc, tc, x_d, wq_d, wk_d, wv_d, wo_d, out_d, bcast_ap,
    ka_sb, mq_sb, om_sb, bq_sb, bk_sb, bv_row, bv_bc, u_row, u_bc,
    scratch, consts, big, wpool, epool, rbpool, rpool, cnpool, opool,
    drampool, pmm, pscore, pctx, ka_d, mq_d, om_d, bq_d, bk_d, bv_d, u_d,
):
    # ---- persistent big tensors (all bf16) ----
    xT = big.tile([P, DC * S], BF16, tag="xT", name="xT").rearrange(
        "p (c s) -> p c s", c=DC
    )
    qT = big.tile([P, HKC * S], BF16, tag="qT", name="qT").rearrange(
        "p (c s) -> p c s", c=HKC
    )
    kT = big.tile([P, HKC * S], BF16, tag="kT", name="kT").rearrange(
        "p (c s) -> p c s", c=HKC
    )
    vext = big.tile([P, SC * H * (K + 1)], BF16, tag="vext", name="vext").rearrange(
        "p (s h k) -> p s h k", s=SC, h=H
    )
    ctxT = big.tile([P, HKC * S], BF16, tag="ctxT", name="ctxT").rearrange(
        "p (c s) -> p c s", c=HKC
    )
    # full-row weight layouts [p = row%128, chunk = row//128, 1024] (2KB lines)
    wqs = wpool.tile([P, DC * HK], BF16, tag="wq", name="wqs").rearrange(
        "p (c m) -> p c m", c=DC
    )
    wks = wpool.tile([P, DC * HK], BF16, tag="wk", name="wks").rearrange(
        "p (c m) -> p c m", c=DC
    )
    wvs = wpool.tile([P, DC * HK], BF16, tag="wv", name="wvs").rearrange(
        "p (c m) -> p c m", c=DC
    )
    wos = wpool.tile([P, HKC * D], BF16, tag="wo", name="wos").rearrange(
        "p (c m) -> p c m", c=HKC
    )

    # ---- phase 0: x -> xT via hardware XBAR DMA transpose + wv, chunked
    # per dc and interleaved across BOTH hardware DGE queues (SP + ACT) so
    # the v-projection matmul for chunk dc can start as soon as (xT[dc],
    # wv[dc]) land, instead of waiting ~23us for monolithic transfers. ----
    # x (pre-transposed on host) + wv, chunked per dc and interleaved
    # across BOTH hardware DGE queues so the v-projection matmul for
    # chunk dc can start as soon as (xT[dc], wv[dc]) land (~10us).
    for dc in range(DC):
        q = nc.sync if dc % 2 == 0 else nc.scalar
        q.dma_start(xT[:, dc, :], x_d.ap()[dc * P : (dc + 1) * P, :])
        q.dma_start(wvs[:, dc, :], wv_d.ap()[dc * P : (dc + 1) * P, :])

    # ones column of vext via Pool-engine memset (a broadcast DMA here
    # generates 16K 2-byte descriptors and stalls the SP queue for >100us)
    nc.gpsimd.memset(
        vext[:, :, :, K : K + 1].rearrange("p a b o -> p (a b) o"), 1.0
    )

    # constants: small LINEAR DMAs only (a broadcast DMA is a ~5us
    # descriptor storm that starves the shared ~170GB/s HBM path right
    # when the x/wv trickle needs it — measured: the v-proj bias stalled
    # to 27-32us on bv_bc). Rows land in <1us; the Pool engine (idle at
    # this point) does the partition broadcasts on-chip.
    nc.sync.dma_start(bv_row[:], bcast_ap(bv_d, [1], [1, HK]))
    nc.sync.dma_start(bq_sb[:], bq_d.ap().rearrange("(p c) -> p c", p=P))
    nc.sync.dma_start(bk_sb[:], bk_d.ap().rearrange("(p c) -> p c", p=P))
    nc.sync.dma_start(ka_sb[:], ka_d.ap().rearrange("(p c) -> p c", p=P))
    nc.sync.dma_start(mq_sb[:], mq_d.ap().rearrange("(p c) -> p c", p=P))
    nc.sync.dma_start(om_sb[:], om_d.ap().rearrange("(p c) -> p c", p=P))
    nc.scalar.dma_start(u_row[:], bcast_ap(u_d, [1], [1, D]))
    nc.gpsimd.partition_broadcast(bv_bc[:], bv_row[:])
    nc.gpsimd.partition_broadcast(u_bc[:, 0:512], u_row[:, 0:512])
    nc.gpsimd.partition_broadcast(u_bc[:, 512:1024], u_row[:, 512:1024])

    # remaining weights, chunked: wq on the ACT queue (behind its half of
    # x/wv), wk+wo on the SP queue. Each lands well before its first
    # consumer (q proj chunk 0 at ~37us, k at ~40us, out proj much later).
    for h4 in range(2):
        nc.scalar.dma_start(
            wqs[:, 4 * h4 : 4 * h4 + 4, :],
            wq_d.ap()[h4 * 512 : (h4 + 1) * 512, :].rearrange(
                "(c p) m -> p c m", p=P
            ),
        )
    for h4 in range(2):
        nc.sync.dma_start(
            wks[:, 4 * h4 : 4 * h4 + 4, :],
            wk_d.ap()[h4 * 512 : (h4 + 1) * 512, :].rearrange(
                "(c p) m -> p c m", p=P
            ),
        )
    for h4 in range(2):
        nc.sync.dma_start(
            wos[:, 4 * h4 : 4 * h4 + 4, :],
            wo_d.ap()[h4 * 512 : (h4 + 1) * 512, :].rearrange(
                "(c p) m -> p c m", p=P
            ),
        )

    # ---- PE warm-up / trickle-filler dummies (results never read):
    # memset scratch is ready ~6us; ~10 dummies trip the HAM activity
    # monitor (clock 1.2 -> 2.4GHz) before the first real matmul at
    # ~10us; 3 more after each first-group chunk keep the activity
    # window busy through the ~1.3us/chunk DMA arrival trickle. ----
    nc.gpsimd.memset(scratch[:], 0.0)

    def dummy_mms(n):
        for _ in range(n):
            pd = pscore.tile([P, 1024], F32, tag="score", name="dummy")
            nc.tensor.matmul(
                pd[:, 0:512], scratch[:, 0:128], scratch[:],
                start=True, stop=True,
            )

    dummy_mms(10)

    # ---- phase 1a: v projection into vext ----
    # First SIX groups (hh=0, st=0..5) run dc-MAJOR with their PSUM
    # accumulators spread across pmm+pscore+pctx (all idle here): each
    # arriving 256KB (xT,wv) chunk releases 6 matmuls instead of 1-2, so
    # the PE rides the ~170GB/s DMA trickle at ~70% busy instead of ~25%
    # (pmm alone allows only 2 groups in flight).
    ps6 = []
    for g in range(6):
        pool = (pmm, pscore, pctx)[g % 3]
        if pool is pscore:
            t = pool.tile([P, 1024], F32, tag="score", name="psv")[:, 0:512]
        elif pool is pctx:
            t = pool.tile([P, 512], F32, tag="ctx", name="psv")[:]
        else:
            t = pool.tile([P, 512], F32, tag="mm", name="ps")[:]
        ps6.append(t)
    for dc in range(DC):
        for g in range(6):
            nc.tensor.matmul(
                ps6[g],
                xT[:, dc, g * P : (g + 1) * P],
                wvs[:, dc, 0:512],
                start=(dc == 0),
                stop=(dc == DC - 1),
            )
        if dc < 7:
            dummy_mms(2)
    for g in range(6):
        nc.vector.tensor_tensor(
            vext[:, g, 0:8, 0:K],
            ps6[g].rearrange("p (h k) -> p h k", k=K),
            bv_bc[:, 0:512].rearrange("p (h k) -> p h k", k=K),
            OP.add,
        )
    for hh in range(2):  # remaining groups, data fully resident by now
        for st in range(SC):
            if hh == 0 and st < 6:
                continue
            ps = pmm.tile([P, 512], F32, tag="mm", name="ps")
            for dc in range(DC):
                nc.tensor.matmul(
                    ps[:],
                    xT[:, dc, st * P : (st + 1) * P],
                    wvs[:, dc, hh * 512 : (hh + 1) * 512],
                    start=(dc == 0),
                    stop=(dc == DC - 1),
                )
            nc.vector.tensor_tensor(
                vext[:, st, hh * 8 : (hh + 1) * 8, 0:K],
                ps[:].rearrange("p (h k) -> p h k", k=K),
                bv_bc[:, hh * 512 : (hh + 1) * 512].rearrange(
                    "p (h k) -> p h k", k=K
                ),
                OP.add,
            )

    # ---- qk projection steps (emitted interleaved with attention) ----
    # matmul computes lhsT.T @ rhs: for qT [hk, s] use lhsT = W chunk
    # [d, hk-cols], rhs = xT [d, s].
    def proj_chunk_steps(hkc):
        steps = []
        for w_sb, b_sb, dst in ((wqs, bq_sb, qT), (wks, bk_sb, kT)):
            for qw in range(NQW):
                ps_box = []

                def alloc(ps_box=ps_box):
                    ps_box.append(pmm.tile([P, 512], F32, tag="mm", name="ps"))

                steps.append(alloc)
                for dc in range(DC):
                    def mm(dc=dc, w_sb=w_sb, qw=qw, hkc=hkc, ps_box=ps_box):
                        nc.tensor.matmul(
                            ps_box[0][:],
                            w_sb[:, dc, hkc * P : (hkc + 1) * P],
                            xT[:, dc, qw * 512 : (qw + 1) * 512],
                            start=(dc == 0),
                            stop=(dc == DC - 1),
                        )
                    steps.append(mm)

                def bias(b_sb=b_sb, dst=dst, qw=qw, hkc=hkc, ps_box=ps_box):
                    nc.vector.tensor_scalar_add(
                        dst[:, hkc, qw * 512 : (qw + 1) * 512],
                        ps_box[0][:],
                        b_sb[:, hkc : hkc + 1],
                    )
                steps.append(bias)
        return steps

    # ---- output-projection step for one (qt, dh): 8 matmuls + blend ----
    # qt < 4 reads only the qw0 half of ctxT (query rows < 512), so those
    # chunks can interleave into chunk 7's qw1 attention — keeping the PE
    # busy across the attention->projection transition (otherwise a ~6us
    # PE gap lets the HAM clock-gate re-throttle to 1.2 GHz for the tail).
    def outproj_acc(po, qt, dh, c_from, c_to):
        for c in range(c_from, c_to):
            nc.tensor.matmul(
                po,
                ctxT[:, c, qt * P : (qt + 1) * P],
                wos[:, c, dh * 512 : (dh + 1) * 512],
                start=(c == 0),
                stop=(c == HKC - 1),
            )

    def outproj_fin(po, qt, dh, via_act):
        # out = (po - (u+bo))*mq + (u+bo)  ==  po*mq + (u+bo)*(1-mq)
        ub = u_bc[:, dh * 512 : (dh + 1) * 512]
        tb = opool.tile([P, 512], BF16, tag="ob", name="tb")
        if via_act:
            # final phase: ACT is idle (no more exps), DVE is the tail
            # bottleneck -> compute both blend products on ACT (scale
            # is per-partition), leaving DVE just the add.
            aa = opool.tile([P, 512], F32, tag="o1", name="aa")
            nc.scalar.activation(
                aa[:], po, AF.Copy, scale=mq_sb[:, qt : qt + 1]
            )
            bb = opool.tile([P, 512], F32, tag="o1", name="bb")
            nc.scalar.activation(
                bb[:], ub, AF.Copy, scale=om_sb[:, qt : qt + 1]
            )
            nc.vector.tensor_tensor(tb[:], aa[:], bb[:], OP.add)
        else:
            t1 = opool.tile([P, 512], F32, tag="o1", name="t1")
            nc.vector.tensor_tensor(t1[:], po, ub, OP.subtract)
            nc.vector.scalar_tensor_tensor(
                tb[:], t1[:], mq_sb[:, qt : qt + 1], ub, OP.mult, OP.add
            )
        nc.sync.dma_start(
            out_d.ap()[qt * P : (qt + 1) * P, dh * 512 : (dh + 1) * 512],
            tb[:],
        )

    def outproj_step(qt, dh, via_act=False):
        def f():
            po = pmm.tile([P, 512], F32, tag="mm", name="ps")
            outproj_acc(po[:], qt, dh, 0, HKC)
            outproj_fin(po[:], qt, dh, via_act)
        return f

    # ---- phase 1b: qk chunk 0 emitted directly ----
    for step in proj_chunk_steps(0):
        step()

    # ---- phase 2: attention per head pair, proj chunk hc+1 interleaved ----
    for hc in range(HKC):
        if hc + 1 < HKC:
            steps_all = proj_chunk_steps(hc + 1)
            half = (len(steps_all) + 1) // 2
            pending_by_qw = [steps_all[:half], steps_all[half:]]
        else:
            pending_by_qw = [
                [],
                [outproj_step(qt, dh) for qt in range(4) for dh in range(2)],
            ]

        hA, hB = 2 * hc, 2 * hc + 1
        for qw in range(NQW):
            pending = pending_by_qw[qw]
            # chunks processed in GROUPS of 2: [4 score mms (64x128 mode)]
            # [proj steps + 4 ctx mms (128x128 mode)] per group — one PE
            # tiling-mode switch per slot instead of two (each switch costs
            # a ~100ns array drain).
            ngroups = SC // 2 + 1
            per_grp = (len(pending) + ngroups - 1) // ngroups
            pi = 0
            pcA = pctx.tile([P, 512], F32, tag="ctx", name="pcA")
            pcB = pctx.tile([P, 512], F32, tag="ctx", name="pcB")
            exs = {}
            for g in range(ngroups):
                if g < SC // 2:
                    for sc in (2 * g, 2 * g + 1):
                        pss2 = pscore.tile(
                            [P, 1024], F32, tag="score", name="pss2"
                        )
                        nc.tensor.matmul(
                            pss2[:, 0:512],
                            kT[0:64, hc, sc * P : (sc + 1) * P],
                            qT[0:64, hc, qw * 512 : (qw + 1) * 512],
                            start=True,
                            stop=True,
                        )
                        nc.tensor.matmul(
                            pss2[:, 512:1024],
                            kT[64:128, hc, sc * P : (sc + 1) * P],
                            qT[64:128, hc, qw * 512 : (qw + 1) * 512],
                            start=True,
                            stop=True,
                        )
                        ex = epool.tile([P, 1024], BF16, tag="exp", name="ex")
                        nc.scalar.activation(
                            ex[:], pss2[:], AF.Exp,
                            bias=ka_sb[:, sc : sc + 1], scale=1.0,
                        )
                        exs[sc] = ex
                # interleaved proj/upath steps (cover ACT latency)
                for _ in range(per_grp):
                    if pi < len(pending):
                        pending[pi]()
                        pi += 1
                if g >= 1:
                    for sc in (2 * g - 2, 2 * g - 1):
                        exm = exs.pop(sc)
                        nc.tensor.matmul(
                            pcA[0:65, :],
                            vext[:, sc, hA, :],
                            exm[:, 0:512],
                            start=(sc == 0),
                            stop=(sc == SC - 1),
                        )
                        nc.tensor.matmul(
                            pcB[0:65, :],
                            vext[:, sc, hB, :],
                            exm[:, 512:1024],
                            start=(sc == 0),
                            stop=(sc == SC - 1),
                        )
            while pi < len(pending):
                pending[pi]()
                pi += 1
            # ---- normalization (off the PSUM critical path): copy each
            # [65,512] accumulator to SBUF first — frees the PSUM bank for
            # the next q-window earlier, and feeds reciprocal_approx_fast
            # from SBUF (from PSUM the bit-trick seed reads garbage on HW).
            def norm_even():
                pcsA = cnpool.tile([64, 512], F32, tag="pcs", name="pcsA")
                nc.vector.tensor_copy(pcsA[:], pcA[0:64, :])
                sumA = rpool.tile([1, 512], F32, tag="rp", name="sumA")
                nc.vector.tensor_copy(sumA[:], pcA[64:65, :])
                recipA = rpool.tile([1, 512], F32, tag="rp", name="recipA")
                nc.vector.reciprocal_approx_fast(recipA[:], sumA[:])
                rbA = rbpool.tile([64, 512], F32, tag="rb", name="rbA")
                nc.gpsimd.partition_broadcast(rbA[:], recipA[:])
                nc.vector.tensor_tensor(
                    ctxT[0:64, hc, qw * 512 : (qw + 1) * 512],
                    pcsA[:],
                    rbA[:],
                    OP.mult,
                )

            def norm_odd(dma_q):
                # odd head: scratch + partition-shift DMA
                pcsB = cnpool.tile([64, 512], F32, tag="pcs", name="pcsB")
                nc.vector.tensor_copy(pcsB[:], pcB[0:64, :])
                sumB = rpool.tile([1, 512], F32, tag="rp", name="sumB")
                nc.vector.tensor_copy(sumB[:], pcB[64:65, :])
                recipB = rpool.tile([1, 512], F32, tag="rp", name="recipB")
                nc.vector.reciprocal_approx_fast(recipB[:], sumB[:])
                rbB = rbpool.tile([64, 512], F32, tag="rb", name="rbB")
                nc.gpsimd.partition_broadcast(rbB[:], recipB[:])
                cnB = cnpool.tile([64, 512], BF16, tag="cn", name="cnB")
                nc.vector.tensor_tensor(cnB[:], pcsB[:], rbB[:], OP.mult)
                dma_q.dma_start(
                    ctxT[64:128, hc, qw * 512 : (qw + 1) * 512], cnB[:]
                )

            if hc == HKC - 1 and qw == NQW - 1:
                # last window: the odd-head partition-shift DMA gates the
                # final out-projection's c=7 matmuls — run the B chain
                # FIRST and dispatch its DMA on the (now idle) ACT queue
                # so it lands ~3us earlier and HAM never re-throttles.
                norm_odd(nc.scalar)
                norm_even()
            else:
                norm_even()
                norm_odd(nc.sync)

    # ---- phase 3: remaining output projection (qt 4-7 need qw1 ctxT).
    # The hc=7/qw=1 normalization chain (DVE copy/recip + Pool broadcast +
    # mult) takes ~5us after the last ctx matmul; only the c=7 matmul of
    # each step depends on it. Pre-accumulate c=0..6 for the first 4 steps
    # (2 po tiles in pmm + 2 riding the now-idle pscore bufs) so the PE
    # stays busy through the chain instead of stalling + HAM-rethrottling.
    finals = [(qt, dh) for qt in range(4, SC) for dh in range(2)]
    pre = []
    for i in range(4):
        qt, dh = finals[i]
        if i % 2 == 0:
            po = pmm.tile([P, 512], F32, tag="mm", name="ps")[:]
        else:
            po = pscore.tile([P, 1024], F32, tag="score", name="po2")[
                :, 0:512
            ]
        outproj_acc(po, qt, dh, 0, HKC - 1)
        pre.append(po)
    for i, (qt, dh) in enumerate(finals):
        if i < 4:
            po = pre[i]
            outproj_acc(po, qt, dh, HKC - 1, HKC)
            outproj_fin(po, qt, dh, via_act=True)
        else:
            # rotate across pmm+pscore (4 po tiles in flight) so the
            # ACT/DVE blend pipeline never gates the matmuls
            if i % 2 == 0:
                po = pmm.tile([P, 512], F32, tag="mm", name="ps")[:]
            else:
                po = pscore.tile([P, 1024], F32, tag="score", name="po2")[
                    :, 0:512
                ]
            outproj_acc(po, qt, dh, 0, HKC)
            outproj_fin(po, qt, dh, via_act=True)


def _get_nc():
    global _nc_cache
    if _nc_cache is None:
        _nc_cache = _build_nc()
    return _nc_cache


_nc_bench_cache = {}


def _get_bench_nc(repeat):
    if repeat not in _nc_bench_cache:
        _nc_bench_cache[repeat] = _build_nc(repeat)
    return _nc_bench_cache[repeat]


def _prep_in_maps(input_tensor, input_mask, Wq, bq, Wk, bk, Wv, bv, Wo, bo):
    bf16 = ml_dtypes.bfloat16
    x = np.ascontiguousarray(np.asarray(input_tensor, dtype=np.float32))
    mask = np.asarray(input_mask).astype(bool)
    Wq = np.asarray(Wq, dtype=np.float32).reshape(D, HK)
    Wk = np.asarray(Wk, dtype=np.float32).reshape(D, HK)
    Wv = np.asarray(Wv, dtype=np.float32).reshape(D, HK)
    Wo = np.asarray(Wo, dtype=np.float32).reshape(HK, D)
    bq = np.asarray(bq, dtype=np.float32).reshape(HK)
    bk = np.asarray(bk, dtype=np.float32).reshape(HK)
    bv = np.asarray(bv, dtype=np.float32).reshape(HK)
    bo = np.asarray(bo, dtype=np.float32).reshape(D)

    # fold the 1/sqrt(K)=1/8 score scale into Wq/bq (exact: power of two)
    wqs = np.ascontiguousarray((Wq / 8.0).astype(bf16))
    bqs = bq / 8.0
    wkb = np.ascontiguousarray(Wk.astype(bf16))
    wvb = np.ascontiguousarray(Wv.astype(bf16))
    wob = np.ascontiguousarray(Wo.astype(bf16))

    mf = mask.astype(np.float32)
    ka = (mf - 1.0) * 1e9   # 0 where kept, -1e9 where masked

    def perm(v):
        # [n*128] chunk-major -> per-partition-contiguous [(p c)] layout
        return np.ascontiguousarray(v.reshape(-1, P).T).reshape(-1)

    bqp = perm(bqs)
    bkp = perm(bk)

    xb = x.astype(bf16)

    # uniform-attention row for fully-masked queries, computed exactly on
    # the host: u = (mean_s(x) @ Wv + bv) @ Wo + bo  (per batch item)
    xmean = x.astype(np.float64).mean(axis=1)            # [B, D]
    u_all = (
        (xmean @ Wv.astype(np.float64) + bv) @ Wo.astype(np.float64) + bo
    ).astype(np.float32)                                 # [B, D]

    in_maps = []
    for b in range(B):
        in_maps.append(
            {
                # host-side transpose: the kernel consumes xT [D, S]
                "x": np.ascontiguousarray(xb[b].T),
                "wq": wqs,
                "wk": wkb,
                "wv": wvb,
                "wo": wob,
                "bq": bqp,
                "bk": bkp,
                "bv": np.ascontiguousarray(bv),
                "u": np.ascontiguousarray(u_all[b]),
                "ka": perm(ka[b]),
                "mq": perm(mf[b]),
                "om": perm(1.0 - mf[b]),
            }
        )
    return in_maps


def kernel(input_tensor, input_mask, Wq, bq, Wk, bk, Wv, bv, Wo, bo):
    in_maps = _prep_in_maps(
        input_tensor, input_mask, Wq, bq, Wk, bk, Wv, bv, Wo, bo
    )
    nc = _get_nc()
    res = run_bass_kernel_spmd(nc, in_maps, core_ids=list(range(B)), trace=TRACE)
    if TRACE:
        kernel.last_result = res
    out = np.stack([r["out"] for r in res.results], axis=0).astype(np.float32)
    return out

